# revision 32
# baseline (speedup 1.0000x reference)
"""Fused transformer block (LN1 -> causal MHA -> residual -> LN2 -> FFN -> residual)
for Trainium2, distributed over 8 NeuronCores: tensor-parallel attention heads
within each batch (4 cores/batch, 4 heads/core), AllToAll of attention outputs,
then sequence-sharded Wo + FFN (512 tokens/core, full W1/W2).

Schedule notes (v4):
- Phase 1 (LN1/transpose/QKV) is FUSED with attention: after chunk n's K/Q
  projection, all score blocks for q-chunk n are emitted, so the Act engine
  (softmax exp, the attention bottleneck) works from ~10us onward instead of
  idling during phase 1.
- Act engine runs ONLY Exp / Sqrt / Copy / Relu, grouped so activation-table
  reloads are rare (ln/exp reciprocal chains are gone: softmax denominators
  use the custom-DVE reciprocal_approx_fast).
- Diagonal-block masks on gpsimd (SBUF-only), normalize-mults + copies on DVE.
- AllToAll (not AllGather) moves only the needed 512-token slices: 4x less
  collective traffic; big weights (w1/wo) stream on the sync queue during
  attention.
- PSUM: 2 shared util banks (transpose/V/KQ rotation) + 4 score banks +
  2 AV banks = exactly 8.
Matmuls in bf16 with fp32 PSUM; LN/softmax math fp32.
"""

import sys

import numpy as np

if "/opt/trn_rl_repo" not in sys.path:
    sys.path.insert(0, "/opt/trn_rl_repo")

import ml_dtypes

B, T, D = 2, 2048, 1024
H, HS = 16, 64
F = 4 * D
NCORES = 8
NH = 4            # heads per core
NF = NH * HS      # 256 features per core
TQ = T // 4       # 512 tokens per core after the exchange
EPS = 1e-5
GROUPS = [[0, 1, 2, 3], [4, 5, 6, 7]]

BF16 = ml_dtypes.bfloat16

_CACHE = {}


def _build(flags):
    """Build the Bass program (identical for all cores). flags: (has_bo, has_b2)."""
    import concourse.bass as bass
    import concourse.mybir as mybir
    import concourse.tile as tile
    from concourse import bacc
    from concourse.bass import ts
    from concourse.masks import make_identity

    has_bo, has_b2 = flags
    f32 = mybir.dt.float32
    bf16 = mybir.dt.bfloat16
    Alu = mybir.AluOpType
    Act = mybir.ActivationFunctionType

    nc = bacc.Bacc("TRN2", target_bir_lowering=False, debug=False, num_devices=8)

    KT = T // 128      # 16 token tiles
    DC = D // 128      # 8 feature chunks of the model dim
    FC = F // 128      # 32 hidden chunks
    QS = TQ // 128     # 4 token tiles per 512-chunk

    # ---- DRAM I/O (host pre-lays weights partition-major/contiguous) ----
    x_full = nc.dram_tensor("x_full", [T, D], bf16, kind="ExternalInput").ap()
    wq = nc.dram_tensor("wq", [128, DC, NF], bf16, kind="ExternalInput").ap()
    wk = nc.dram_tensor("wk", [128, DC, NF], bf16, kind="ExternalInput").ap()
    wv = nc.dram_tensor("wv", [128, DC, NF], bf16, kind="ExternalInput").ap()
    wo = nc.dram_tensor("wo", [128, DC, D], bf16, kind="ExternalInput").ap()
    w1 = nc.dram_tensor("w1", [128, DC, F], bf16, kind="ExternalInput").ap()
    w2 = nc.dram_tensor("w2", [F, D], bf16, kind="ExternalInput").ap()
    b1d = nc.dram_tensor("b1", [128, FC], f32, kind="ExternalInput").ap()
    bod = nc.dram_tensor("bo", [D], f32, kind="ExternalInput").ap() if has_bo else None
    b2d = nc.dram_tensor("b2", [D], f32, kind="ExternalInput").ap() if has_b2 else None
    out = nc.dram_tensor("out", [TQ, D], f32, kind="ExternalOutput").ap()

    ag_in = [nc.dram_tensor(f"ag_in{kq}", [128, T], bf16, kind="Internal").ap()
             for kq in range(2)]
    ag_out = [nc.dram_tensor(f"ag_out{kq}", [4, 128, T], bf16,
                             kind="Internal").ap()
              for kq in range(2)]

    with tile.TileContext(nc) as tc:
        with (
            tc.tile_pool(name="const", bufs=1) as cst,
            tc.tile_pool(name="w1P", bufs=1) as w1P,
            tc.tile_pool(name="actB2", bufs=1) as actB2,
        ):
            # --- constants ---
            ident = cst.tile([128, 128], bf16)
            make_identity(nc, ident)
            eps_t = cst.tile([128, 1], f32)
            nc.vector.memset(eps_t, EPS)
            zero_t = cst.tile([128, 1], f32)
            nc.vector.memset(zero_t, 0.0)
            b1_sb = cst.tile([128, FC], f32)
            nc.scalar.dma_start(out=b1_sb, in_=b1d)
            if has_bo:
                bo_b = cst.tile([128, D], f32)
                nc.scalar.dma_start(
                    out=bo_b,
                    in_=bass.AP(tensor=bod.tensor, offset=bod.offset,
                                ap=[[0, 128]] + list(bod.ap)))
            if has_b2:
                b2_b = cst.tile([128, D], f32)
                nc.scalar.dma_start(
                    out=b2_b,
                    in_=bass.AP(tensor=b2d.tensor, offset=b2d.offset,
                                ap=[[0, 128]] + list(b2d.ap)))

            # persistent across FFN: W1 resident + post-attention activations
            w1_sb = w1P.tile([128, DC, F], bf16, name="w1sb")
            x2_sb = [actB2.tile([128, D], bf16, name=f"x2{i}")
                     for i in range(QS)]
            h2_fm = [actB2.tile([128, TQ], bf16, name=f"h2f{d}")
                     for d in range(DC)]

            rq = nc.sync.partition_id() % 4

            # ---- mid pool: lives through phases 1-4, freed before FFN ----
            with tc.tile_pool(name="mid", bufs=1) as midP:
                # early small weights on the act queue (cheap contiguous
                # descs), before any activations are enqueued there
                wvc = midP.tile([128, DC, NF], bf16, name="wvc")
                nc.scalar.dma_start(out=wvc, in_=wv)
                wkc = midP.tile([128, DC, NF], bf16, name="wkc")
                nc.scalar.dma_start(out=wkc, in_=wk)
                wqc = midP.tile([128, DC, NF], bf16, name="wqc")
                nc.scalar.dma_start(out=wqc, in_=wq)
                wo_sb = midP.tile([128, DC, D], bf16, name="wosb")

                q_fm = [midP.tile([128, T], bf16, name=f"qfm{m}")
                        for m in range(2)]
                k_fm = [midP.tile([128, T], bf16, name=f"kfm{m}")
                        for m in range(2)]
                v_sb = [midP.tile([128, NH, HS + 1], bf16, name=f"vsb{t}")
                        for t in range(KT)]
                attnT = [midP.tile([128, T], bf16, name=f"at{d}")
                         for d in range(2)]
                h_fm = midP.tile([128, DC, T], bf16, name="hfm")

                # == Fused phases 1-3: LN1/transpose/QKV interleaved with ===
                # == attention blocks (per 512-token chunk)               ===
                with (
                    tc.tile_pool(name="ph12", bufs=2) as ph12,
                    tc.tile_pool(name="ph4", bufs=6) as ph4,
                    tc.tile_pool(name="smm", bufs=2) as smm,
                    tc.tile_pool(name="psU", bufs=2, space="PSUM") as psU,
                    tc.tile_pool(name="psS", bufs=2, space="PSUM") as psS,
                    tc.tile_pool(name="psAV", bufs=1, space="PSUM") as psAV,
                ):
                    LAG = 6

                    for t in range(KT):
                        nc.gpsimd.memset(v_sb[t][:, :, HS:HS + 1], 1.0)

                    def emit_av(ent):
                        kq, pav, pk, first, last, pe, qi = ent
                        for sub in range(2):
                            nc.tensor.matmul(pav[:, ts(sub, TQ)],
                                             v_sb[pk][:, 2 * kq + sub, :],
                                             pe[:, ts(sub, TQ)],
                                             start=first, stop=last)
                        if last:
                            # per-q denominators -> reciprocal on DVE
                            den = smm.tile([1, 2 * TQ], f32, tag="den",
                                           name="den", bufs=1)
                            nc.vector.tensor_copy(out=den,
                                                  in_=pav[HS:HS + 1, :])
                            dr = smm.tile([1, 2 * TQ], f32, tag="dr",
                                          name="dr", bufs=1)
                            nc.vector.reciprocal_approx_fast(out=dr, in_=den)
                            for sub in range(2):
                                bc = smm.tile([HS, TQ], f32, tag="bc",
                                              name="bc")
                                nc.gpsimd.partition_broadcast(
                                    bc, dr[0:1, ts(sub, TQ)])
                                nc.vector.tensor_tensor(
                                    out=attnT[kq][ts(sub, HS), ts(qi, TQ)],
                                    in0=pav[0:HS, ts(sub, TQ)], in1=bc,
                                    op=Alu.mult)

                    def issue_ag(kq):
                        nc.sync.dma_start(out=ag_in[kq], in_=attnT[kq])
                        nc.gpsimd.collective_compute(
                            "AllGather",
                            mybir.AluOpType.bypass,
                            replica_groups=GROUPS,
                            ins=[ag_in[kq]],
                            outs=[ag_out[kq]],
                        )

                    pending = []

                    def emit_block(kq, i, kb, pav, first, last):
                        pss = psS.tile([128, 2 * TQ], f32, tag="s", name="pss")
                        for sub in range(2):
                            ro = sub * HS
                            nc.tensor.matmul(
                                pss[:, ts(sub, TQ)],
                                k_fm[kq][ro:ro + HS, ts(kb, 128)],
                                q_fm[kq][ro:ro + HS, ts(i, TQ)],
                                start=True, stop=True)
                        et = ph4.tile([128, 2 * TQ], bf16, tag="exp",
                                      name="et")
                        nc.scalar.activation(out=et, in_=pss, func=Act.Exp,
                                             scale=0.125, bias=zero_t)
                        if kb >= 4 * i:
                            # zero the upper-triangle of the diagonal block:
                            # keep where q - (d*128 + key_row) >= 0
                            etv = et.rearrange("p (s q) -> p s q", q=TQ)
                            nc.gpsimd.affine_select(
                                out=etv, in_=etv,
                                pattern=[[0, 2], [1, TQ]],
                                base=-(kb - 4 * i) * 128,
                                channel_multiplier=-1,
                                compare_op=Alu.is_ge,
                                fill=0.0)
                        pending.append((kq, pav, kb, first, last, et, i))
                        if len(pending) > LAG:
                            ent = pending.pop(0)
                            emit_av(ent)
                            if ent[0] == 0 and ent[4] and ent[6] == QS - 1:
                                issue_ag(0)

                    for n in range(QS):
                        for t in range(4 * n, 4 * n + 4):
                            xt = ph12.tile([128, D], bf16, tag="xt",
                                           name="xt", bufs=3)
                            nc.sync.dma_start(out=xt, in_=x_full[ts(t, 128), :])
                            xg = xt.rearrange("p (n f) -> p n f", f=512)
                            stats = ph12.tile([128, 2, 6], f32, tag="st",
                                              name="st")
                            for sg in range(2):
                                nc.vector.bn_stats(out=stats[:, sg, :],
                                                   in_=xg[:, sg, :])
                            mv = ph12.tile([128, 2], f32, tag="mv", name="mv")
                            nc.vector.bn_aggr(out=mv, in_=stats)
                            rstd = ph12.tile([128, 1], f32, tag="rs",
                                             name="rs")
                            nc.scalar.activation(out=rstd, in_=mv[:, 1:2],
                                                 func=Act.Sqrt, bias=eps_t,
                                                 scale=1.0)
                            nc.vector.reciprocal(out=rstd, in_=rstd)
                            ht = ph12.tile([128, D], bf16, tag="ht", name="ht")
                            nc.vector.tensor_scalar(
                                out=ht, in0=xt, scalar1=mv[:, 0:1],
                                scalar2=rstd, op0=Alu.subtract, op1=Alu.mult)
                            for half in range(2):
                                # [128,1024] bf16 = same 2KB slot as the f32
                                # psk/pv allocations sharing tag "u"
                                trv = psU.tile([128, 1024], bf16, tag="u",
                                               name="tr")
                                for j in range(4):
                                    nc.tensor.transpose(
                                        trv[:, ts(j, 128)],
                                        ht[:, ts(half * 4 + j, 128)], ident)
                                nc.vector.tensor_copy(
                                    out=h_fm[:, half * 4:half * 4 + 4,
                                             ts(t, 128)],
                                    in_=trv[:, 0:512].rearrange(
                                        "p (j q) -> p j q", q=128))
                            # V projection for this token tile
                            pv = psU.tile([128, 512], f32, tag="u", name="pv")
                            for k in range(DC):
                                nc.tensor.matmul(pv[:, 0:NF],
                                                 h_fm[:, k, ts(t, 128)],
                                                 wvc[:, k, :],
                                                 start=(k == 0),
                                                 stop=(k == DC - 1))
                            nc.vector.tensor_copy(
                                out=v_sb[t][:, :, 0:HS],
                                in_=pv[:, 0:NF].rearrange("p (h d) -> p h d",
                                                          d=HS))
                        # K and Q projections for chunk n (512 tokens)
                        for fc in range(2):
                            psk = psU.tile([128, TQ], f32, tag="u", name="psk")
                            for k in range(DC):
                                nc.tensor.matmul(psk, wkc[:, k, ts(fc, 128)],
                                                 h_fm[:, k, ts(n, 512)],
                                                 start=(k == 0),
                                                 stop=(k == DC - 1))
                            nc.scalar.copy(out=k_fm[fc][:, ts(n, 512)],
                                           in_=psk)
                            psq = psU.tile([128, TQ], f32, tag="u", name="psq")
                            for k in range(DC):
                                nc.tensor.matmul(psq, wqc[:, k, ts(fc, 128)],
                                                 h_fm[:, k, ts(n, 512)],
                                                 start=(k == 0),
                                                 stop=(k == DC - 1))
                            nc.vector.tensor_copy(out=q_fm[fc][:, ts(n, 512)],
                                                  in_=psq)
                        if n == 0:
                            # big late weights stream on the sync queue while
                            # it is idle (needed from the Wo/FFN phases)
                            for kk in range(DC):
                                nc.sync.dma_start(out=w1_sb[:, kk, :],
                                                  in_=w1[:, kk, :])
                            nc.sync.dma_start(out=wo_sb, in_=wo)
                        # attention blocks for q-chunk n, head-pair 0 only;
                        # pair 1 is deferred so AG#0 hides under its blocks
                        pav = psAV.tile([HS + 1, 2 * TQ], f32, tag="av",
                                        name="pav")
                        nkb = 4 * (n + 1)
                        for kb in range(nkb):
                            emit_block(0, n, kb, pav, kb == 0, kb == nkb - 1)
                    for n in range(QS):
                        pav = psAV.tile([HS + 1, 2 * TQ], f32, tag="av",
                                        name="pav")
                        nkb = 4 * (n + 1)
                        for kb in range(nkb):
                            emit_block(1, n, kb, pav, kb == 0, kb == nkb - 1)
                    for ent in pending:
                        emit_av(ent)
                        if ent[0] == 0 and ent[4] and ent[6] == QS - 1:
                            issue_ag(0)
                    issue_ag(1)

                # ====== Phase 4: Wo (own tokens, exchanged features) + x2 ===
                with (
                    tc.tile_pool(name="zP", bufs=1) as zP,
                    tc.tile_pool(name="ph6", bufs=3) as ph6,
                    tc.tile_pool(name="psO", bufs=4, space="PSUM") as psO,
                    tc.tile_pool(name="psT2", bufs=2, space="PSUM") as psT2,
                ):
                    z_sb = [zP.tile([128, TQ], bf16, name=f"z{j}")
                            for j in range(DC)]
                    xq_sb = [zP.tile([128, D], bf16, name=f"xq{i}")
                             for i in range(QS)]
                    # rank-sliced loads of the gathered blocks (own tokens)
                    for i in range(QS):
                        srcx = bass.AP(
                            tensor=x_full.tensor,
                            offset=rq * (TQ * D) + i * 128 * D,
                            ap=[[D, 128], [1, D]])
                        nc.sync.dma_start(out=xq_sb[i], in_=srcx)
                    for kq in range(2):
                        for s in range(4):
                            srcz = bass.AP(
                                tensor=ag_out[kq].tensor,
                                offset=s * 128 * T + rq * TQ,
                                ap=[[T, 128], [1, TQ]])
                            nc.sync.dma_start(out=z_sb[kq * 4 + s], in_=srcz)
                    # pass 1: first 4 gathered blocks (after AG#0, which is
                    # long done) overlap AG#1's flight
                    for i in range(QS):
                        pso = [psO.tile([128, 512], f32, tag="o", name="pso")
                               for n in range(2)]
                        for j in range(4):
                            for n in range(2):
                                nc.tensor.matmul(pso[n], z_sb[j][:, ts(i, 128)],
                                                 wo_sb[:, j, ts(n, 512)],
                                                 start=(j == 0), stop=(j == 3))
                        for n in range(2):
                            nc.vector.tensor_tensor(
                                out=x2_sb[i][:, ts(n, 512)], in0=pso[n],
                                in1=xq_sb[i][:, ts(n, 512)], op=Alu.add)
                    # pass 2: remaining 4 blocks after AG#1
                    for i in range(QS):
                        pso = [psO.tile([128, 512], f32, tag="o", name="pso")
                               for n in range(2)]
                        for j in range(4, DC):
                            for n in range(2):
                                nc.tensor.matmul(pso[n], z_sb[j][:, ts(i, 128)],
                                                 wo_sb[:, j, ts(n, 512)],
                                                 start=(j == 4),
                                                 stop=(j == DC - 1))
                        for n in range(2):
                            nc.vector.tensor_tensor(
                                out=x2_sb[i][:, ts(n, 512)],
                                in0=x2_sb[i][:, ts(n, 512)],
                                in1=pso[n], op=Alu.add)
                        if has_bo:
                            nc.vector.tensor_tensor(
                                out=x2_sb[i], in0=x2_sb[i], in1=bo_b,
                                op=Alu.add)
                        xg = x2_sb[i].rearrange("p (n f) -> p n f", f=512)
                        stats = ph6.tile([128, 2, 6], f32, tag="st", name="st6")
                        for sg in range(2):
                            nc.vector.bn_stats(out=stats[:, sg, :],
                                               in_=xg[:, sg, :])
                        mv = ph6.tile([128, 2], f32, tag="mv", name="mv6")
                        nc.vector.bn_aggr(out=mv, in_=stats)
                        rstd = ph6.tile([128, 1], f32, tag="rs", name="rs6")
                        nc.scalar.activation(out=rstd, in_=mv[:, 1:2],
                                             func=Act.Sqrt, bias=eps_t,
                                             scale=1.0)
                        nc.vector.reciprocal(out=rstd, in_=rstd)
                        h2t = ph6.tile([128, D], bf16, tag="h2t", name="h2t")
                        nc.vector.tensor_scalar(
                            out=h2t, in0=x2_sb[i], scalar1=mv[:, 0:1],
                            scalar2=rstd, op0=Alu.subtract, op1=Alu.mult)
                        for half in range(2):
                            ps = psT2.tile([128, 512], bf16, tag="tr",
                                           name="tr2")
                            for j in range(4):
                                nc.tensor.transpose(
                                    ps[:, ts(j, 128)],
                                    h2t[:, ts(half * 4 + j, 128)], ident)
                            for j in range(4):
                                nc.vector.tensor_copy(
                                    out=h2_fm[half * 4 + j][:, ts(i, 128)],
                                    in_=ps[:, ts(j, 128)])

            # ================= Phase 6: FFN1 ============================
            with tc.tile_pool(name="g1P", bufs=1) as g1P:
                g1 = [g1P.tile([128, TQ], bf16, name=f"g1t{m}")
                      for m in range(FC)]
                with tc.tile_pool(name="psF", bufs=3, space="PSUM") as psF:
                    for m in range(FC):
                        ps = psF.tile([128, TQ], f32, tag="mm", name="psf")
                        for k in range(DC):
                            nc.tensor.matmul(ps, w1_sb[:, k, ts(m, 128)],
                                             h2_fm[k][:, 0:TQ],
                                             start=(k == 0), stop=(k == DC - 1))
                        nc.scalar.activation(out=g1[m], in_=ps, func=Act.Relu,
                                             bias=b1_sb[:, m:m + 1], scale=1.0)

                # ================= Phase 7: FFN2 ========================
                with (
                    tc.tile_pool(name="ph8", bufs=4) as ph8,
                    tc.tile_pool(name="ph8o", bufs=2) as ph8o,
                    tc.tile_pool(name="ps8", bufs=1, space="PSUM") as ps8,
                ):
                    psum2 = [ps8.tile([128, 512], f32, tag=f"p8_{j}",
                                      name=f"p8_{j}") for j in range(8)]
                    for m in range(FC):
                        w2c = ph8.tile([128, D], bf16, tag="w2c", name="w2c")
                        nc.sync.dma_start(out=w2c, in_=w2[ts(m, 128), :])
                        for i in range(QS):
                            for n in range(2):
                                nc.tensor.matmul(
                                    psum2[i * 2 + n],
                                    g1[m][:, ts(i, 128)],
                                    w2c[:, ts(n, 512)],
                                    start=(m == 0), stop=(m == FC - 1))
                    for i in range(QS):
                        ot = ph8o.tile([128, D], f32, tag="ot", name="ot")
                        for n in range(2):
                            nc.vector.tensor_tensor(
                                out=ot[:, ts(n, 512)],
                                in0=psum2[i * 2 + n],
                                in1=x2_sb[i][:, ts(n, 512)], op=Alu.add)
                        if has_b2:
                            nc.vector.tensor_tensor(
                                out=ot, in0=ot, in1=b2_b, op=Alu.add)
                        nc.sync.dma_start(out=out[ts(i, 128), :], in_=ot)

    nc.compile()
    return nc


def _prep(inputs):
    """Host-side shard prep. Returns in_maps (one dict per core) + flags."""
    x = np.asarray(inputs["x"], np.float32)
    ln1_g = np.asarray(inputs["ln1_g"], np.float32)
    ln1_b = np.asarray(inputs["ln1_b"], np.float32)
    ln2_g = np.asarray(inputs["ln2_g"], np.float32)
    ln2_b = np.asarray(inputs["ln2_b"], np.float32)
    assert np.all(ln1_b == 0.0) and np.all(ln2_b == 0.0), "ln biases must be 0"

    # fold ln gains into the consuming weight matrices
    wq = (ln1_g[:, None] * np.asarray(inputs["Wq"], np.float32)).astype(BF16)
    wk = (ln1_g[:, None] * np.asarray(inputs["Wk"], np.float32)).astype(BF16)
    wv = (ln1_g[:, None] * np.asarray(inputs["Wv"], np.float32)).astype(BF16)
    wo = np.asarray(inputs["Wo"], np.float32).astype(BF16)
    w1 = (ln2_g[:, None] * np.asarray(inputs["W1"], np.float32)).astype(BF16)
    w2 = np.asarray(inputs["W2"], np.float32).astype(BF16)
    b1 = np.asarray(inputs["b1"], np.float32)
    bo = np.asarray(inputs["bo"], np.float32)
    b2 = np.asarray(inputs["b2"], np.float32)
    has_bo = bool(np.any(bo != 0.0))
    has_b2 = bool(np.any(b2 != 0.0))

    def pmajor(w, cols):
        # [D, cols] with D = 8*128 rows -> [128, 8, cols] partition-major
        return np.ascontiguousarray(
            w.reshape(8, 128, cols).transpose(1, 0, 2))

    xb = x.astype(BF16)

    # Wo rows permuted to the exchanged feature order:
    # block j = kq*4 + s holds rank s's pair kq = heads (4s+2kq, 4s+2kq+1)
    # = original Wo rows [s*256 + kq*128 : s*256 + kq*128 + 128).
    wo_p = np.concatenate(
        [wo[s * NF + kq * 128: s * NF + kq * 128 + 128]
         for kq in range(2) for s in range(4)])
    wo_p = pmajor(wo_p, D)
    w1_p = pmajor(w1, F)
    b1_p = np.ascontiguousarray(b1.reshape(32, 128).T)

    in_maps = []
    for c in range(NCORES):
        g, r = c // 4, c % 4
        m = {
            "x_full": np.ascontiguousarray(xb[g]),
            "wq": pmajor(wq[:, r * NF:(r + 1) * NF], NF),
            "wk": pmajor(wk[:, r * NF:(r + 1) * NF], NF),
            "wv": pmajor(wv[:, r * NF:(r + 1) * NF], NF),
            "wo": wo_p,
            "w1": w1_p, "w2": w2, "b1": b1_p,
        }
        if has_bo:
            m["bo"] = bo
        if has_b2:
            m["b2"] = b2
        in_maps.append(m)
    return in_maps, (has_bo, has_b2)


def _run(inputs, profile_dir=None):
    from concourse import bass_utils

    in_maps, flags = _prep(inputs)
    if flags not in _CACHE:
        _CACHE[flags] = _build(flags)
    nc = _CACHE[flags]

    if profile_dir is not None:
        from concourse import bass2jax
        from trn_agent_boot.trn_boot import _ntff_profile_via_ctypes
        hook = _ntff_profile_via_ctypes("/opt/axon/libaxon_pjrt.so")
        with hook(profile_dir, [0]):
            results = bass2jax.run_bass_via_pjrt(nc, in_maps, n_cores=NCORES)
    else:
        res = bass_utils.run_bass_kernel_spmd(
            nc, in_maps, core_ids=list(range(NCORES))
        )
        results = res.results

    out = np.empty((B, T, D), np.float32)
    for c in range(NCORES):
        g, r = c // 4, c % 4
        out[g, r * TQ:(r + 1) * TQ] = results[c]["out"]
    return out


def kernel(**inputs) -> np.ndarray:
    return _run(inputs)
